# revision 1
# baseline (speedup 1.0000x reference)
"""DeformableResidualBlock kernel for 8 Trainium2 NeuronCores.

Decomposition:
  - Host (numpy): im2col stacking, offset-conv coordinate math, bilinear
    corner indexing/weighting (data-dependent gather prep).
  - Device (Bass/Tile, 8 cores): the two big deformable-conv einsums
    out[o,p] = sum_{c,k} w[o,c,k] * sampled[c,k,p]  (K=576 contraction),
    sharded data-parallel over (batch, image half): core i -> (b=i//2, half).

Shapes hardcoded per spec: x [4, 64, 128, 128] f32, K=3 deformable taps.
"""

import numpy as np

import concourse.bacc as bacc
import concourse.mybir as mybir
import concourse.tile as tile
from concourse.bass_utils import run_bass_kernel_spmd

B, C, H, W = 4, 64, 128, 128
KK = 9          # 3x3 taps
NEG = 0.01      # leaky relu slope
HW = H * W
NSH = HW // 2   # pixels per core (half image)
KDIM = C * KK   # 576 contraction
KPAD = 640      # padded to 5 x 128
NCHUNK = 512
F32 = mybir.dt.float32

_CACHED = {}


def _build_nc():
    """One tiled matmul program: out[64, 8192] = w[640, 64]^T @ xs[640, 8192]."""
    nc = bacc.Bacc("TRN2", target_bir_lowering=False, debug=False,
                   enable_asserts=False, num_devices=8)
    w_d = nc.dram_tensor("w", [KPAD, 64], F32, kind="ExternalInput")
    xs_d = nc.dram_tensor("xs", [KPAD, NSH], F32, kind="ExternalInput")
    out_d = nc.dram_tensor("out", [64, NSH], F32, kind="ExternalOutput")

    with tile.TileContext(nc) as tc:
        with (
            tc.tile_pool(name="wp", bufs=1) as wp,
            tc.tile_pool(name="xp", bufs=12) as xp,
            tc.tile_pool(name="pp", bufs=4, space="PSUM") as pp,
            tc.tile_pool(name="op", bufs=4) as op,
        ):
            wts = []
            for ki in range(5):
                wt = wp.tile([128, 64], F32, tag=f"w{ki}")
                nc.sync.dma_start(wt[:], w_d[ki * 128:(ki + 1) * 128, :])
                wts.append(wt)
            for n0 in range(0, NSH, NCHUNK):
                ps = pp.tile([64, NCHUNK], F32)
                for ki in range(5):
                    xt = xp.tile([128, NCHUNK], F32)
                    nc.sync.dma_start(
                        xt[:], xs_d[ki * 128:(ki + 1) * 128, n0:n0 + NCHUNK])
                    nc.tensor.matmul(ps[:], wts[ki][:], xt[:],
                                     start=(ki == 0), stop=(ki == 4))
                ot = op.tile([64, NCHUNK], F32)
                nc.vector.tensor_copy(ot[:], ps[:])
                nc.sync.dma_start(out_d[:, n0:n0 + NCHUNK], ot[:])
    nc.compile()
    return nc


def _device_einsum(w_mat, stacks):
    """w_mat [576, 64]; stacks [B, 576, HW] -> [B, 64, HW] via 8 cores."""
    if "nc" not in _CACHED:
        _CACHED["nc"] = _build_nc()
    nc = _CACHED["nc"]
    wp = np.zeros((KPAD, 64), np.float32)
    wp[:KDIM] = w_mat
    in_maps = []
    for i in range(8):
        b, half = i // 2, i % 2
        xsp = np.zeros((KPAD, NSH), np.float32)
        xsp[:KDIM] = stacks[b, :, half * NSH:(half + 1) * NSH]
        in_maps.append({"w": wp, "xs": np.ascontiguousarray(xsp)})
    res = run_bass_kernel_spmd(nc, in_maps, core_ids=list(range(8)))
    out = np.zeros((B, 64, HW), np.float32)
    for i in range(8):
        b, half = i // 2, i % 2
        out[b, :, half * NSH:(half + 1) * NSH] = res.results[i]["out"]
    return out


def _offsets(x, w_off, b_off):
    """Regular 3x3 offset conv on the CPU jax backend (fast eigen conv)."""
    import jax

    with jax.default_device(jax.devices("cpu")[0]):
        y = jax.jit(
            lambda a, w: jax.lax.conv_general_dilated(
                a, w, (1, 1), [(1, 1), (1, 1)],
                dimension_numbers=("NCHW", "OIHW", "NCHW"))
        )(x, w_off)
    return np.asarray(y) + b_off[None, :, None, None]


def _sample_stack(x, off):
    """Bilinear-gather stack: x [B,C,H,W], off [B,18,H,W] -> [B, C*KK, HW]."""
    off = off.reshape(B, KK, 2, H, W)
    dy, dx = off[:, :, 0], off[:, :, 1]                  # [B, KK, H, W]
    ky, kx = np.meshgrid(np.arange(3), np.arange(3), indexing="ij")
    base_y = (np.arange(H, dtype=np.float32)[None, None, :, None]
              + (ky.reshape(-1).astype(np.float32) - 1)[None, :, None, None])
    base_x = (np.arange(W, dtype=np.float32)[None, None, None, :]
              + (kx.reshape(-1).astype(np.float32) - 1)[None, :, None, None])
    py = base_y + dy
    px = base_x + dx
    y0 = np.floor(py)
    x0 = np.floor(px)
    wy1 = (py - y0).astype(np.float32)
    wx1 = (px - x0).astype(np.float32)
    wy0 = np.float32(1.0) - wy1
    wx0 = np.float32(1.0) - wx1
    import scipy.sparse as sp

    flat = x.reshape(B, C, HW)
    idx_list, wv_list = [], []
    for (yi, xi, wgt) in ((y0, x0, wy0 * wx0), (y0, x0 + 1, wy0 * wx1),
                          (y0 + 1, x0, wy1 * wx0), (y0 + 1, x0 + 1, wy1 * wx1)):
        valid = (yi >= 0) & (yi < H) & (xi >= 0) & (xi < W)
        yc = np.clip(yi, 0, H - 1).astype(np.int32)
        xc = np.clip(xi, 0, W - 1).astype(np.int32)
        idx_list.append((yc * W + xc).reshape(B, -1))    # [B, KK*HW]
        wv_list.append((wgt * valid).astype(np.float32).reshape(B, -1))
    indptr = np.arange(0, 4 * KK * HW + 1, 4, dtype=np.int64)
    out = np.empty((B, KDIM, HW), np.float32)
    for b in range(B):
        indices = np.stack([idx[b] for idx in idx_list], axis=1).ravel()
        data = np.stack([wv[b] for wv in wv_list], axis=1).ravel()
        A = sp.csr_matrix((data, indices, indptr), shape=(KK * HW, HW))
        g = A @ flat[b].T                                # [KK*HW, C]
        out[b] = (g.reshape(KK, HW, C).transpose(2, 0, 1)
                  .reshape(KDIM, HW))
    return out


def _deform_layer(x, w_off, b_off, w, b):
    off = _offsets(x, w_off, b_off)
    stack = _sample_stack(x, off)
    w_mat = np.ascontiguousarray(w.reshape(64, KDIM).T)  # [576, 64]
    y = _device_einsum(w_mat, stack)                     # [B, 64, HW]
    return (y + b[None, :, None]).reshape(B, 64, H, W)


def _leaky(v):
    return np.where(v >= 0, v, np.float32(NEG) * v).astype(np.float32)


def kernel(x, w_off1, b_off1, w1, b1, w_off2, b_off2, w2, b2):
    x = np.asarray(x, np.float32)
    h = _leaky(_deform_layer(x, np.asarray(w_off1, np.float32),
                             np.asarray(b_off1, np.float32),
                             np.asarray(w1, np.float32),
                             np.asarray(b1, np.float32)))
    y = _deform_layer(h, np.asarray(w_off2, np.float32),
                      np.asarray(b_off2, np.float32),
                      np.asarray(w2, np.float32),
                      np.asarray(b2, np.float32))
    return _leaky(y + x)



# revision 34
# speedup vs baseline: 21.6680x; 21.6680x over previous
"""DeformableResidualBlock kernel for Trainium2 (4 active NeuronCores).

Everything runs on-device in one launch per call:
  - offset conv (3x3) as 9 shifted matmuls over a zero-padded image
    (output channels padded 18 -> 41 so dy lands at partition 0 and dx at
    partition 32, respecting the engine partition-quadrant rule)
  - bilinear sample coords/weights with DVE elementwise math in
    quadrant-aligned 9-row slots
  - data-dependent gather via the GPSIMD ap_gather extended instruction
    (indices in its 16-partition-wrapped layout, built with PE transposes
    of the [slot, pixel] tiles + strided fold copies)
  - deformable einsum as per-tap matmuls with transposed [pixel, chan]
    output so bilinear weights apply as per-partition scalars
  - residual + leaky relu fused on-chip

Core b handles sample b fully (batch-parallel over 4 cores). x/y wire
format is fp16. A module-cached jax.jit runner avoids per-call
retracing; only input transfer + execution is paid per call.

Shapes hardcoded per spec: x [4, 64, 128, 128] f32.
"""

import numpy as np

import concourse.bacc as bacc
import concourse.mybir as mybir
import concourse.tile as tile
from concourse import library_config

B, C, H, W = 4, 64, 128, 128
PW = W + 2          # padded width
NEG = 0.01
F32 = mybir.dt.float32
F16 = mybir.dt.float16
I16 = mybir.dt.int16
AL = mybir.AluOpType
ACT = mybir.ActivationFunctionType

_CACHED = {}


def _build_nc(HR):
    """Program for one core: full deformable residual block on one sample.

    HR = image rows (128 full size; smaller for sim tests). HR % 8 == 0.
    """
    PH = HR + 2
    NPIX = HR * W
    NCH = HR // 8          # chunks of 8 rows = 1024 px
    PIMG = PH * PW         # padded image elems

    nc = bacc.Bacc("TRN2", target_bir_lowering=False, debug=False,
                   enable_asserts=False, num_devices=8)

    x_d = nc.dram_tensor("x", [C, NPIX], F16, kind="ExternalInput")
    wof1_d = nc.dram_tensor("wof1", [C, 9 * 41], F32, kind="ExternalInput")
    wof2_d = nc.dram_tensor("wof2", [C, 9 * 41], F32, kind="ExternalInput")
    w1_d = nc.dram_tensor("w1", [C, 9 * 64], F32, kind="ExternalInput")
    w2_d = nc.dram_tensor("w2", [C, 9 * 64], F32, kind="ExternalInput")
    bp_d = nc.dram_tensor("bp", [1, 128], F32, kind="ExternalInput")
    bof_d = nc.dram_tensor("bof", [9, 4], F32, kind="ExternalInput")
    kyx_d = nc.dram_tensor("kyx", [9, 2], F32, kind="ExternalInput")
    idn_d = nc.dram_tensor("idn", [128, 128], F32, kind="ExternalInput")
    y_d = nc.dram_tensor("y", [C, NPIX], F16, kind="ExternalOutput")

    with tile.TileContext(nc) as tc:
        with (
            tc.tile_pool(name="pers", bufs=1) as pers,
            tc.tile_pool(name="ost", bufs=1) as ost,
            tc.tile_pool(name="sm", bufs=1) as sm,
            tc.tile_pool(name="poff", bufs=1, space="PSUM") as poff,
            tc.tile_pool(name="poff2", bufs=1, space="PSUM") as poff2,
            tc.tile_pool(name="pT", bufs=1, space="PSUM") as pT,
            tc.tile_pool(name="pm", bufs=2, space="PSUM") as pm,
            tc.tile_pool(name="pc", bufs=1, space="PSUM") as pc,
        ):
            xp = pers.tile([C, PH, PW], F32, tag="xp")
            hpad = pers.tile([C, PH, PW], F32, tag="hpad")
            wof1 = pers.tile([C, 9 * 41], F32, tag="wof1")
            wof2 = pers.tile([C, 9 * 41], F32, tag="wof2")
            w1 = pers.tile([C, 9 * 64], F32, tag="w1")
            w2 = pers.tile([C, 9 * 64], F32, tag="w2")
            bpt = pers.tile([1, 128], F32, tag="bpt")
            bofyx = pers.tile([9, 4], F32, tag="bofyx")
            bofc = pers.tile([9, 1], F32, tag="bofc")
            kyxt = pers.tile([9, 2], F32, tag="kyxt")
            idn = pers.tile([128, 128], F32, tag="idn")
            ones1 = pers.tile([1, 128], F32, tag="ones1")
            b1rep = pers.tile([128, 64], F32, tag="b1rep")
            b2rep = pers.tile([128, 64], F32, tag="b2rep")
            # SBUF scratch, 9-row quadrant slots at partitions 0/32/64/96
            rampY = pers.tile([128, 1024], F32, tag="rampY")  # rampY,cb1,x0f,y0f
            rampX = pers.tile([128, 1024], F32, tag="rampX")  # rampX,py,px,tmp
            cbw = pers.tile([128, 1024], F32, tag="cbw")  # wx1,wx0,cb0,wy1s
            idx4 = pers.tile([128, 1024], F32, tag="idx4")
            wt4 = pers.tile([128, 1024], F32, tag="wt4")
            w16 = pers.tile([64, 4, 9, 64], I16, tag="w16")
            wtT = pers.tile([128, 288], F32, tag="wtT")
            acc = pers.tile([128, 512], F32, tag="acc")
            g = pers.tile([C, 9216], F32, tag="g")

            # ---- setup ----
            nc.sync.dma_start(wof1[:], wof1_d[:])
            nc.sync.dma_start(wof2[:], wof2_d[:])
            nc.sync.dma_start(w1[:], w1_d[:])
            nc.sync.dma_start(w2[:], w2_d[:])
            nc.sync.dma_start(bpt[:], bp_d[:])
            nc.sync.dma_start(bofyx[:], bof_d[:])
            nc.sync.dma_start(kyxt[:], kyx_d[:])
            nc.sync.dma_start(idn[:], idn_d[:])

            nc.vector.memset(xp[:], 0.0)
            nc.vector.memset(hpad[:], 0.0)
            nc.vector.memset(idx4[:], 0.0)
            nc.vector.memset(wt4[:], 0.0)
            # stage fp16 x through a bitcast view of g, then cast-copy into
            # the padded fp32 image interior
            xstage = g[:].bitcast(F16)[:, 0:NPIX]
            nc.sync.dma_start(xstage, x_d[:])
            nc.scalar.copy(xp[:, 1:HR + 1, 1:W + 1],
                           xstage.rearrange("c (h w) -> c h w", h=HR))

            # base coordinate ramps in padded coords, [9, 1024] each:
            # rampY[k, p] = p // 128 + ky(k); rampX[k, p] = p % 128 + kx(k)
            nc.gpsimd.iota(rampY[0:9, :], pattern=[[1, 8], [0, 128]], base=0,
                           channel_multiplier=0,
                           allow_small_or_imprecise_dtypes=True)
            nc.gpsimd.iota(rampX[0:9, :], pattern=[[0, 8], [1, 128]], base=0,
                           channel_multiplier=0,
                           allow_small_or_imprecise_dtypes=True)
            nc.vector.tensor_scalar(rampY[0:9, :], rampY[0:9, :],
                                    kyxt[:, 0:1], None, AL.add)
            nc.vector.tensor_scalar(rampX[0:9, :], rampX[0:9, :],
                                    kyxt[:, 1:2], None, AL.add)

            nc.vector.memset(ones1[:], 1.0)
            pb = pm.tile([128, 64], F32, tag="mp")
            nc.tensor.matmul(pb[:], ones1[:], bpt[:, 0:64], start=True, stop=True)
            nc.scalar.copy(b1rep[:], pb[:])
            pb2 = pm.tile([128, 64], F32, tag="mp")
            nc.tensor.matmul(pb2[:], ones1[:], bpt[:, 64:128], start=True, stop=True)
            nc.scalar.copy(b2rep[:], pb2[:])

            nc.gpsimd.load_library(library_config.ap_gather)

            def layer(src, wof, wmat, bof_col, second):
                for cc in range(NCH):
                    y0 = cc * 8
                    # ---- offset conv: dy rows 0-8, dx rows 32-40 (PSUM A) ----
                    A = poff.tile([128, 1024], F32, tag="A")
                    for s2 in range(2):
                        r0 = y0 + s2 * 4
                        for k9 in range(9):
                            ky, kx = k9 // 3, k9 % 3
                            nc.tensor.matmul(
                                A[0:41, s2 * 512:(s2 + 1) * 512],
                                wof[:, k9 * 41:(k9 + 1) * 41],
                                src[:, r0 + ky:r0 + ky + 4, kx:kx + W],
                                start=(k9 == 0), stop=(k9 == 8))

                    # ---- coords: intermediates live in PSUM A/B quadrant
                    # slots so every tensor_tensor has at most one SBUF input
                    # and all partition starts are 0/32/64/96 ----
                    B = poff2.tile([128, 1024], F32, tag="B")
                    nc.vector.tensor_scalar(
                        bofc[:], bofyx[:, 2 * bof_col:2 * bof_col + 1],
                        float(y0), None, AL.add)
                    # every DVE op touches at most ONE PSUM operand; SB-SB
                    # input pairs share a base partition; floor is done by
                    # biasing +16384 and round-tripping through int32 (the
                    # bias is folded into all downstream clip/scale consts)
                    BI = 16384.0
                    py, px = rampX[32:41, :], rampX[64:73, :]     # SBUF
                    nc.vector.scalar_tensor_tensor(
                        py, A[0:9, :], bofc[:], rampY[0:9, :], AL.add, AL.add)
                    nc.vector.scalar_tensor_tensor(
                        px, A[32:41, :],
                        bofyx[:, 2 * bof_col + 1:2 * bof_col + 2],
                        rampX[0:9, :], AL.add, AL.add)
                    yb, xb = B[0:9, :], B[32:41, :]               # PSUM
                    nc.vector.tensor_scalar(yb, py, BI, None, AL.add)
                    nc.vector.tensor_scalar(xb, px, BI, None, AL.add)
                    yi = rampY[64:73, :].bitcast(mybir.dt.int32)  # SBUF i32
                    xi = rampY[96:105, :].bitcast(mybir.dt.int32)
                    nc.vector.tensor_copy(yi, yb)
                    nc.vector.tensor_copy(xi, xb)
                    y0fb, x0fb = B[64:73, :], B[96:105, :]        # PSUM f32
                    nc.vector.tensor_copy(y0fb, yi)
                    nc.vector.tensor_copy(x0fb, xi)
                    # exact floor fixup: the f32->i32 cast rounding mode is
                    # trunc in sim but nearest on hw; subtract 1 wherever the
                    # cast landed above py/px
                    gty = rampY[64:73, :]              # f32 view (yi dead)
                    gtx = rampY[96:105, :]
                    nc.vector.scalar_tensor_tensor(
                        gty, y0fb, -BI, py, AL.add, AL.is_gt)
                    nc.vector.scalar_tensor_tensor(
                        gtx, x0fb, -BI, px, AL.add, AL.is_gt)
                    y0fb2 = rampY[32:41, :]            # SBUF, same base as py
                    x0fb2 = cbw[64:73, :]              # SBUF, same base as px
                    nc.vector.tensor_sub(y0fb2, y0fb, gty)
                    nc.vector.tensor_sub(x0fb2, x0fb, gtx)
                    # wy1n = floor(py) - py = -wy1 (exact)
                    wy1n = cbw[96:105, :]
                    nc.vector.scalar_tensor_tensor(
                        wy1n, y0fb2, -BI, py, AL.add, AL.subtract)
                    wx1n = cbw[0:9, :]
                    nc.vector.scalar_tensor_tensor(
                        wx1n, x0fb2, -BI, px, AL.add, AL.subtract)
                    wy1p, wy0p = A[64:73, :], A[96:105, :]        # PSUM
                    nc.vector.tensor_scalar(wy1p, wy1n, -1.0, None, AL.mult)
                    nc.vector.tensor_scalar(wy0p, wy1n, 1.0, None, AL.add)
                    wx1 = cbw[32:41, :]
                    wx0 = rampX[96:105, :]
                    nc.vector.tensor_scalar(wx1, wx1n, -1.0, None, AL.mult)
                    nc.vector.tensor_scalar(wx0, wx1n, 1.0, None, AL.add)
                    # row/col bases, biased by +16384 throughout
                    tmpyb = rampY[64:73, :]            # gty dead
                    nc.vector.tensor_scalar(tmpyb, y0fb2, BI - 1.0, None, AL.max)
                    rb1b = rampX[32:41, :]             # py dead
                    nc.vector.tensor_scalar(rb1b, tmpyb, float(HR) + BI,
                                            None, AL.min)
                    rm1 = B[0:9, :]                    # PSUM (yb dead)
                    nc.vector.tensor_scalar(rm1, rb1b, 130.0,
                                            -((BI - 1.0) * 130.0 + BI),
                                            AL.mult, AL.add)
                    rb0b = rampY[64:73, :]             # tmpyb dead
                    nc.vector.tensor_scalar(rb0b, y0fb2, BI + float(HR + 1),
                                            BI, AL.min, AL.max)
                    rm0 = B[32:41, :]                  # PSUM (xb dead)
                    nc.vector.tensor_scalar(rm0, rb0b, 130.0,
                                            -(BI * 130.0 + BI),
                                            AL.mult, AL.add)
                    cb0b = cbw[96:105, :]              # wy1n dead
                    nc.vector.tensor_scalar(cb0b, x0fb2, BI + 129.0, BI,
                                            AL.min, AL.max)
                    tmpxb = rampX[64:73, :]            # px dead
                    nc.vector.tensor_scalar(tmpxb, x0fb2, BI - 1.0, None, AL.max)
                    cb1b = rampY[32:41, :]             # y0fb2 dead
                    nc.vector.tensor_scalar(cb1b, tmpxb, BI + 128.0, 1.0,
                                            AL.min, AL.add)
                    # idx4 slots: 00@0, 01@32, 10@64, 11@96 (PSUM rm + SB cb)
                    nc.vector.tensor_add(idx4[0:9, :], rm0, cb0b)
                    nc.vector.tensor_add(idx4[32:41, :], rm0, cb1b)
                    nc.vector.tensor_add(idx4[64:73, :], rm1, cb0b)
                    nc.vector.tensor_add(idx4[96:105, :], rm1, cb1b)
                    nc.vector.tensor_mul(wt4[0:9, :], wy0p, wx0)
                    nc.vector.tensor_mul(wt4[32:41, :], wy0p, wx1)
                    nc.vector.tensor_mul(wt4[64:73, :], wy1p, wx0)
                    nc.vector.tensor_mul(wt4[96:105, :], wy1p, wx1)

                    # ---- wrap idx to gpsimd layout + transpose weights ----
                    # two transposes per pixel tile: the second input window
                    # is shifted 16 pixels so odd wrap-groups land at
                    # quadrant-aligned partitions (engine reads need
                    # partition starts 0/32/64/96)
                    for t8 in range(8):
                        base = t8 * 128
                        psT = pT.tile([128, 4, 32], F32, tag="psT")
                        nc.tensor.transpose(
                            psT[:], idx4[:, base:base + 128], idn[:])
                        for v, u in enumerate((0, 2, 4, 6)):
                            c0 = t8 * 8 + u
                            nc.vector.tensor_copy(
                                w16[0:16, :, :, c0:c0 + 1],
                                psT[v * 32:v * 32 + 16, :, 0:9])
                        psT2 = pT.tile([128, 4, 32], F32, tag="psT")
                        nc.tensor.transpose(
                            psT2[0:112, :, :], idx4[:, base + 16:base + 128],
                            idn[:])
                        for v, u in enumerate((1, 3, 5, 7)):
                            c0 = t8 * 8 + u
                            nc.vector.tensor_copy(
                                w16[0:16, :, :, c0:c0 + 1],
                                psT2[v * 32:v * 32 + 16, :, 0:9])
                        psW = pT.tile([128, 4, 32], F32, tag="psT")
                        nc.tensor.transpose(
                            psW[:], wt4[:, base:base + 128], idn[:])
                        nc.scalar.copy(wtT[:, t8 * 36:(t8 + 1) * 36],
                                       psW[:, :, 0:9])
                    # replicate wrapped idx to all four 16-partition groups
                    nc.sync.dma_start(w16[16:32, :, :, :], w16[0:16, :, :, :])
                    nc.sync.dma_start(w16[32:64, :, :, :], w16[0:32, :, :, :])

                    # ---- gather + weighted einsum ----
                    nc.vector.memset(acc[:], 0.0)
                    for cr in range(4):
                        nc.gpsimd.ap_gather(
                            g[:], src[:], w16[:, cr, :, :],
                            channels=64, num_elems=PIMG, d=1, num_idxs=9216)
                        for k9 in range(9):
                            for t8 in range(8):
                                mp = pm.tile([128, 64], F32, tag="mp")
                                nc.tensor.matmul(
                                    mp[:],
                                    g[:, k9 * 1024 + t8 * 128:
                                       k9 * 1024 + (t8 + 1) * 128],
                                    wmat[:, k9 * 64:(k9 + 1) * 64],
                                    start=True, stop=True)
                                a = acc[:, t8 * 64:(t8 + 1) * 64]
                                nc.vector.scalar_tensor_tensor(
                                    a, mp[:],
                                    wtT[:, t8 * 36 + cr * 9 + k9:
                                        t8 * 36 + cr * 9 + k9 + 1],
                                    a, AL.mult, AL.add)

                    # ---- finalize chunk ----
                    if second:
                        ystage = ost.tile([C, 1024], F16, tag="ystage")
                    for t8 in range(8):
                        yg = y0 + t8
                        vsb = sm.tile([128, 64], F32, tag="vsb")
                        brep = b2rep if second else b1rep
                        nc.vector.tensor_add(
                            vsb[:], acc[:, t8 * 64:(t8 + 1) * 64], brep[:])
                        if not second:
                            hT = sm.tile([128, 64], F32, tag="hT")
                            nc.vector.scalar_tensor_tensor(
                                hT[:], vsb[:], NEG, vsb[:], AL.mult, AL.max)
                            pct = pc.tile([64, 128], F32, tag="pct")
                            nc.tensor.transpose(pct[:], hT[:], idn[:])
                            nc.scalar.copy(hpad[:, yg + 1, 1:W + 1], pct[:])
                        else:
                            pct = pc.tile([64, 128], F32, tag="pct")
                            nc.tensor.transpose(pct[:], vsb[:], idn[:])
                            ysl = ystage[:, t8 * 128:(t8 + 1) * 128]
                            nc.vector.tensor_add(
                                ysl, pct[:], xp[:, yg + 1, 1:W + 1])
                            nc.vector.scalar_tensor_tensor(
                                ysl, ysl, NEG, ysl, AL.mult, AL.max)
                    if second:
                        nc.sync.dma_start(
                            y_d[:, cc * 1024:(cc + 1) * 1024], ystage[:])

            layer(xp, wof1, w1, 0, False)
            layer(hpad, wof2, w2, 1, True)

    nc.compile()
    return nc


# ---------------- host side ----------------

def _prep_weights(w_off, b_off, w):
    """Host packing for one layer: wofT [64, 9*41], wT [64, 9*64], bof [41]."""
    perm = list(range(0, 18, 2)) + list(range(1, 18, 2))  # dy taps | dx taps
    wof_p = np.asarray(w_off, np.float32)[perm]           # [18, 64, 3, 3]
    # pad output channels 18 -> 41: dy at rows 0-8, dx at rows 32-40
    wof_pad = np.zeros((41, C, 3, 3), np.float32)
    wof_pad[0:9] = wof_p[0:9]
    wof_pad[32:41] = wof_p[9:18]
    wofT = np.ascontiguousarray(
        wof_pad.transpose(1, 2, 3, 0).reshape(C, 9 * 41))
    wT = np.ascontiguousarray(
        np.asarray(w, np.float32).transpose(1, 2, 3, 0).reshape(C, 9 * 64))
    bof_p = np.asarray(b_off, np.float32)[perm]
    return wofT, wT, bof_p[0:9], bof_p[9:18]


def _get_runner(nc, n_cores):
    """Cached jitted PJRT runner (mirrors bass2jax.run_bass_via_pjrt)."""
    import jax
    from jax.sharding import Mesh, PartitionSpec
    from jax.experimental.shard_map import shard_map
    from concourse.bass2jax import (_bass_exec_p, install_neuronx_cc_hook,
                                    partition_id_tensor)
    import concourse.mybir as mb

    install_neuronx_cc_hook()
    assert nc.dbg_addr is None
    pid_name = nc.partition_id_tensor.name if nc.partition_id_tensor else None

    in_names, out_names, out_avals, zero_shapes = [], [], [], []
    for alloc in nc.m.functions[0].allocations:
        if not isinstance(alloc, mb.MemoryLocationSet):
            continue
        name = alloc.memorylocations[0].name
        if alloc.kind == "ExternalInput":
            if name != pid_name:
                in_names.append(name)
        elif alloc.kind == "ExternalOutput":
            shape = tuple(alloc.tensor_shape)
            dtype = mb.dt.np(alloc.dtype)
            out_names.append(name)
            out_avals.append(jax.core.ShapedArray(shape, dtype))
            zero_shapes.append((shape, dtype))
    n_params = len(in_names)
    n_outs = len(out_avals)
    all_names = in_names + out_names
    if pid_name is not None:
        all_names = all_names + [pid_name]
    donate = tuple(range(n_params, n_params + n_outs))

    def _body(*args):
        operands = list(args)
        if pid_name is not None:
            operands.append(partition_id_tensor())
        outs = _bass_exec_p.bind(
            *operands,
            out_avals=tuple(out_avals),
            in_names=tuple(all_names),
            out_names=tuple(out_names),
            lowering_input_output_aliases=(),
            sim_require_finite=True,
            sim_require_nnan=True,
            nc=nc,
        )
        return tuple(outs)

    devices = jax.devices()[:n_cores]
    mesh = Mesh(np.asarray(devices), ("core",))
    in_specs = (PartitionSpec("core"),) * (n_params + n_outs)
    out_specs = (PartitionSpec("core"),) * n_outs
    sharded = jax.jit(
        shard_map(_body, mesh=mesh, in_specs=in_specs, out_specs=out_specs,
                  check_rep=False),
        donate_argnums=donate, keep_unused=True)

    def run(in_maps):
        concat_in = [
            np.concatenate([m[name] for m in in_maps], axis=0)
            for name in in_names
        ]
        concat_zeros = [
            np.zeros((n_cores * s[0], *s[1:]), dt) for s, dt in zero_shapes
        ]
        out_arrs = sharded(*concat_in, *concat_zeros)
        return [
            {name: np.asarray(out_arrs[i]).reshape(n_cores, *out_avals[i].shape)[c]
             for i, name in enumerate(out_names)}
            for c in range(n_cores)
        ]

    return run


def kernel(x, w_off1, b_off1, w1, b1, w_off2, b_off2, w2, b2):
    if "nc" not in _CACHED:
        _CACHED["nc"] = _build_nc(H)
        _CACHED["run"] = _get_runner(_CACHED["nc"], B)
    run = _CACHED["run"]

    x = np.asarray(x, np.float32)
    wof1T, w1T, bofy1, bofx1 = _prep_weights(w_off1, b_off1, w1)
    wof2T, w2T, bofy2, bofx2 = _prep_weights(w_off2, b_off2, w2)
    bp = np.concatenate([np.asarray(b1, np.float32),
                         np.asarray(b2, np.float32)])[None, :]
    bof = np.stack([bofy1, bofx1, bofy2, bofx2], axis=1).astype(np.float32)
    ky, kx = np.meshgrid(np.arange(3), np.arange(3), indexing="ij")
    kyx = np.stack([ky.reshape(-1), kx.reshape(-1)], axis=1).astype(np.float32)
    idn = np.eye(128, dtype=np.float32)

    x16 = x.reshape(B, C, H * W).astype(np.float16)
    shared = {"wof1": wof1T, "wof2": wof2T, "w1": w1T, "w2": w2T,
              "bp": bp, "bof": bof, "kyx": kyx, "idn": idn}
    in_maps = [{"x": np.ascontiguousarray(x16[b]), **shared} for b in range(B)]
    res = run(in_maps)
    y = np.stack([res[b]["y"].astype(np.float32) for b in range(B)])
    return y.reshape(B, C, H, W)
